# revision 1
# baseline (speedup 1.0000x reference)
"""MoE model (embed -> gate -> 4 dense experts -> softmax combine) on 8 TRN2 cores.

Data-parallel: batch (65536 tokens) sharded 8192/core; expert/gating weights
replicated on every core (SBUF-resident, bf16). All on-chip activations are
kept feature-major ("transposed") so that every matmul consumes operands in
their natural layout:

  e_T[f, t]   = embedding lookup, feature-major, via transposing gather DMAs
                issued one supertile ahead on the otherwise-idle GpSimd SWDGE
                path (fallback: one-hot-mask matmul on the PE).
  h_T[d, t]   = silu(W1[e].T-tiles @ e_T + b1)       (PSUM fp32, evac bf16)
  eo_T[o, t]  = W2[e].T-tiles @ h_T + b2             (PSUM fp32)
  logits[e,t] = Wg.T-tiles @ e_T + bg ; softmax via exp / sum (unnormalized
                weights combined first, one reciprocal row scale at the end)
  out_T[o, t] = (sum_e exp_e * eo_e) * recip         (DVE, fp32)

Output per core is [128, 8192] (feature-major); host transposes on unshard.

bf16 inputs with fp32 PSUM accumulation: end-to-end relative error vs the
fp32 reference is ~0.5%.
"""

import os
import numpy as np
import ml_dtypes

import concourse.bass as bass
import concourse.mybir as mybir
import concourse.tile as tile
from concourse.bass_utils import run_bass_kernel_spmd

BF16 = ml_dtypes.bfloat16

B = 65536
V = 512
D = 1024
IN = 2048
E = 4
OUT = 128
NCORES = 8
BL = B // NCORES          # tokens per core
ST = 512                  # tokens per supertile (max PSUM free dim, fp32)
NST = BL // ST            # supertiles per core
KC = IN // 128            # 16 feature chunks
DC = D // 128             # 8 hidden chunks
VC = V // 128             # 4 vocab chunks

LAST_EXEC_NS = None       # set when BASSMOE_TRACE=1


def _legalize_waits(nc, max_waits=1):
    """This walrus build rejects instructions carrying more than ~1 sync-wait
    command ("Too many sync wait commands", CoreV2/V3GenImpl setupSyncWait).
    Hoist all but the last wait of every instruction onto single-wait NoOps
    placed immediately before it in the same engine's stream."""
    for f in nc.m.functions:
        for bb in f.blocks:
            insts = bb.instructions
            if not any(
                inst.sync_info is not None and len(inst.sync_info.on_wait) > max_waits
                for inst in insts
            ):
                continue
            new = []
            for inst in insts:
                si = inst.sync_info
                waits = list(si.on_wait) if si is not None else []
                if len(waits) > max_waits:
                    for w in waits[:-max_waits]:
                        nop = mybir.InstNoOp(
                            name=f"legw-{nc.next_id()}", ins=[], outs=[]
                        )
                        nop.engine = inst.engine
                        nop.sync_info = mybir.SyncInfo(on_wait=[w], on_update=[])
                        new.append(nop)
                    inst.sync_info = mybir.SyncInfo(
                        on_wait=waits[-max_waits:], on_update=list(si.on_update)
                    )
                new.append(inst)
            bb.instructions = new


def build_program(nst=NST, legalize=True, n_gather=2):
    """n_gather: how many of the 2 embedding tables use the gather-DMA path
    (the rest use the one-hot matmul path)."""
    dt = mybir.dt
    f32, bf16, f16 = dt.float32, dt.bfloat16, dt.float16
    AF = mybir.ActivationFunctionType
    ALU = mybir.AluOpType

    gathered = [t < n_gather for t in range(2)]
    n_onehot = 2 - n_gather

    nc = bass.Bass()

    xd = [None, None]
    for t in range(2):
        if gathered[t]:
            # wrapped gather-idx layout: idx j at [j%16, j//16], replicated
            # across the 8 gpsimd cores
            xd[t] = nc.dram_tensor(
                f"x{t}i", [nst, 128, ST // 16], dt.int16, kind="ExternalInput"
            )
        else:
            xd[t] = nc.dram_tensor(
                f"x{t}", [nst, 1, ST], f16, kind="ExternalInput"
            )
    if n_gather:
        embgd = nc.dram_tensor("embg", [n_gather, V, D], bf16, kind="ExternalInput")
    if n_onehot:
        embd = nc.dram_tensor(
            "embs", [128, n_onehot, VC, DC, 128], bf16, kind="ExternalInput"
        )
        ivd = nc.dram_tensor("ivs", [128, VC], f32, kind="ExternalInput")
    w1d = nc.dram_tensor("w1s", [E, 128, KC, DC, 128], bf16, kind="ExternalInput")
    w2d = nc.dram_tensor("w2s", [128, E, DC, OUT], bf16, kind="ExternalInput")
    wgd = nc.dram_tensor("wgs", [128, KC, E], bf16, kind="ExternalInput")
    b1d = nc.dram_tensor("b1s", [128, E, DC], f32, kind="ExternalInput")
    b2d = nc.dram_tensor("b2s", [128, E], f32, kind="ExternalInput")
    bgd = nc.dram_tensor("bgs", [E, 1], f32, kind="ExternalInput")
    seld = nc.dram_tensor("sels", [E, E, 128], bf16, kind="ExternalInput")
    outd = nc.dram_tensor("out", [128, nst * ST], f32, kind="ExternalOutput")

    with tile.TileContext(nc) as tc:
        with (
            tc.tile_pool(name="const", bufs=1) as cpool,
            tc.tile_pool(name="xt", bufs=2) as xpool,
            tc.tile_pool(name="mask", bufs=1) as mpool,
            tc.tile_pool(name="etg", bufs=2) as etgpool,
            tc.tile_pool(name="et", bufs=1) as etpool,
            tc.tile_pool(name="hs", bufs=1) as hpool,
            tc.tile_pool(name="sm", bufs=2) as smpool,
            tc.tile_pool(name="gsc", bufs=1) as gspool,
            tc.tile_pool(name="sgp", bufs=2) as sgpool,
            tc.tile_pool(name="accp", bufs=2) as apool,
            tc.tile_pool(name="outp", bufs=2) as opool,
            tc.tile_pool(name="pmm", bufs=2, space="PSUM") as pmm,
            tc.tile_pool(name="peo", bufs=2, space="PSUM") as peo,
            tc.tile_pool(name="prb", bufs=2, space="PSUM") as prb,
            tc.tile_pool(name="pmisc", bufs=2, space="PSUM") as pmisc,
        ):
            # --- prologue: supertile 0's embedding inputs first ---
            if n_gather:
                from concourse import library_config

                nc.gpsimd.load_library(library_config.mlp)

                def issue_gather(i, t):
                    """table t embedding rows for supertile i -> feature-major
                    e_T chunk tile, via the GpSimd transposing gather DMA."""
                    xi = xpool.tile([128, ST // 16], dt.int16, tag=f"xi{t}")
                    nc.sync.dma_start(xi[:], xd[t][i])
                    etg = etgpool.tile([128, DC, ST], bf16, tag=f"eTg{t}")
                    nc.gpsimd.dma_gather(
                        out_ap=etg[:],
                        in_ap=embgd[t],
                        idxs_ap=xi[:],
                        num_idxs=ST,
                        num_idxs_reg=ST,
                        elem_size=D,
                        transpose=True,
                    )
                    return etg

            if n_onehot:
                iv_sb = cpool.tile([128, VC], f32)
                nc.sync.dma_start(iv_sb[:], ivd[:])
                ones_f16 = cpool.tile([1, 128], f16)
                nc.vector.memset(ones_f16[:], 1.0)
                x0_pre = []
                for t in range(2):
                    if not gathered[t]:
                        xs = xpool.tile([1, ST], f16, tag=f"x{t}")
                        nc.sync.dma_start(xs[:], xd[t][0])
                        x0_pre.append(xs)
                emb_sb = cpool.tile([128, n_onehot, VC, DC, 128], bf16)
                nc.sync.dma_start(emb_sb[:], embd[:])

            cur_etg = [issue_gather(0, t) if gathered[t] else None for t in range(2)]

            # --- resident weights (DMA queue order = when they are needed) ---
            wg_sb = cpool.tile([128, KC, E], bf16)
            nc.sync.dma_start(wg_sb[:], wgd[:])
            b1_sb = cpool.tile([128, E, DC], f32)
            nc.sync.dma_start(b1_sb[:], b1d[:])
            b2_sb = cpool.tile([128, E], f32)
            nc.sync.dma_start(b2_sb[:], b2d[:])
            bg_sb = cpool.tile([E, 1], f32)
            nc.sync.dma_start(bg_sb[:], bgd[:])
            sel_sb = cpool.tile([E, E, 128], bf16)
            nc.sync.dma_start(sel_sb[:], seld[:])
            w1_sbs = []
            for e in range(E):
                t = cpool.tile([128, KC, DC, 128], bf16, tag=f"w1e{e}")
                w1_sbs.append(t)
            nc.sync.dma_start(w1_sbs[0][:], w1d[0])
            w2_sb = cpool.tile([128, E, DC, OUT], bf16)
            nc.sync.dma_start(w2_sb[:], w2d[:])
            for e in range(1, E):
                nc.sync.dma_start(w1_sbs[e][:], w1d[e])

            ones4_bf = cpool.tile([E, 1], bf16)
            nc.vector.memset(ones4_bf[:], 1.0)
            ones128_bf = cpool.tile([1, 128], bf16)
            nc.vector.memset(ones128_bf[:], 1.0)

            def build_masks(i, preloaded=None):
                """x-broadcast (K=1 matmul) + one-hot compares for the
                one-hot-embedded tables of supertile i."""
                ms = {}
                pi = 0
                for t in range(2):
                    if gathered[t]:
                        continue
                    if preloaded is None:
                        xs = xpool.tile([1, ST], f16, tag=f"x{t}")
                        nc.sync.dma_start(xs[:], xd[t][i])
                    else:
                        xs = preloaded[pi]
                        pi += 1
                    p = pmisc.tile([128, ST], f32, tag="misc")
                    nc.tensor.matmul(p[:], ones_f16[:], xs[:])
                    row = []
                    for vc in range(VC):
                        m = mpool.tile([128, ST], bf16, tag=f"m{t}{vc}")
                        nc.vector.tensor_scalar(
                            m[:], p[:], iv_sb[:, vc : vc + 1], None, ALU.is_equal
                        )
                        row.append(m)
                    ms[t] = row
                return ms

            cur_masks = build_masks(0, preloaded=x0_pre) if n_onehot else {}

            for i in range(nst):
                # --- one-hot embedding matmul -> e_T (one-hot tables) ---
                if n_onehot:
                    eT = etpool.tile([128, n_onehot, DC, ST], bf16, tag="eT")
                    oh = 0
                    for t in range(2):
                        if gathered[t]:
                            continue
                        for dc in range(DC):
                            ps = pmm.tile([128, ST], f32, tag="mm")
                            for vc in range(VC):
                                nc.tensor.matmul(
                                    ps[:],
                                    emb_sb[:, oh, vc, dc, :],
                                    cur_masks[t][vc][:],
                                    start=(vc == 0),
                                    stop=(vc == VC - 1),
                                )
                            nc.scalar.copy(eT[:, oh, dc, :], ps[:])
                        oh += 1

                oh_index = {}
                oh = 0
                for t in range(2):
                    if not gathered[t]:
                        oh_index[t] = oh
                        oh += 1

                def eT_chunk(kc):
                    t, dc = kc // DC, kc % DC
                    if gathered[t]:
                        return cur_etg[t][:, dc, :]
                    return eT[:, oh_index[t], dc, :]

                # --- gating: logits -> exp -> sum -> reciprocal bcast ---
                lp = pmisc.tile([E, ST], f32, tag="misc")
                for kc in range(KC):
                    nc.tensor.matmul(
                        lp[:],
                        wg_sb[:, kc, :],
                        eT_chunk(kc),
                        start=(kc == 0),
                        stop=(kc == KC - 1),
                    )
                expt = smpool.tile([E, ST], bf16, tag="expt")
                nc.scalar.activation(expt[:], lp[:], AF.Exp, bias=bg_sb[:])

                def emit_recip_chain():
                    # sum-exp -> reciprocal -> bf16 -> broadcast to 128 rows.
                    # Emitted between expert 0 and 1 so the slow single-
                    # partition RECIPROCAL (~3.3us DVE) and the Exp/Sigmoid
                    # ACT-table switch hide under expert-0's W1 matmuls
                    # instead of stalling the PE at the supertile boundary.
                    sp = pmisc.tile([1, ST], f32, tag="misc")
                    nc.tensor.matmul(sp[:], ones4_bf[:], expt[:])
                    rec = smpool.tile([1, ST], f32, tag="rec")
                    nc.vector.reciprocal(rec[:], sp[:])
                    recb = smpool.tile([1, ST], bf16, tag="recb")
                    nc.vector.tensor_copy(recb[:], rec[:])
                    rbp = prb.tile([128, ST], f32, tag="rb")
                    nc.tensor.matmul(rbp[:], ones128_bf[:], recb[:])
                    return rbp

                # prefetch next supertile's embeddings: gather DMAs + mask
                # compares overlap with the expert phase below
                next_etg = [None, None]
                if i + 1 < nst:
                    for t in range(2):
                        if gathered[t]:
                            next_etg[t] = issue_gather(i + 1, t)
                    next_masks = build_masks(i + 1) if n_onehot else {}

                # --- experts ---
                acc = apool.tile([128, ST], f32, tag="acc")
                for e in range(E):
                    if e == 1:
                        rbp = emit_recip_chain()
                    # hs as per-chunk tiles: W2's dc-th matmul then only waits
                    # for the dc-th silu chunk, not the whole expert's h
                    hs = []
                    for dc in range(DC):
                        hp = pmm.tile([128, ST], f32, tag="mm")
                        for kc in range(KC):
                            nc.tensor.matmul(
                                hp[:],
                                w1_sbs[e][:, kc, dc, :],
                                eT_chunk(kc),
                                start=(kc == 0),
                                stop=(kc == KC - 1),
                            )
                        sg = sgpool.tile([128, ST], f32, tag="sg")
                        nc.scalar.activation(
                            sg[:], hp[:], AF.Sigmoid, bias=b1_sb[:, e, dc : dc + 1]
                        )
                        h_dc = hpool.tile([128, ST], bf16, tag=f"hs{dc}")
                        nc.vector.scalar_tensor_tensor(
                            h_dc[:], hp[:], b1_sb[:, e, dc : dc + 1], sg[:],
                            ALU.add, ALU.mult,
                        )
                        hs.append(h_dc)
                    eop = peo.tile([128, ST], f32, tag="eo")
                    for dc in range(DC):
                        nc.tensor.matmul(
                            eop[:],
                            w2_sb[:, e, dc, :],
                            hs[dc][:],
                            start=(dc == 0),
                            stop=(dc == DC - 1),
                        )
                    gp = pmisc.tile([128, ST], f32, tag="misc")
                    nc.tensor.matmul(gp[:], sel_sb[:, e, :], expt[:])
                    gs = gspool.tile([128, ST], f32, tag="gs")
                    nc.scalar.copy(gs[:], gp[:])
                    if e == 0:
                        nc.vector.scalar_tensor_tensor(
                            acc[:], eop[:], b2_sb[:, e : e + 1], gs[:],
                            ALU.add, ALU.mult,
                        )
                    else:
                        tmp = opool.tile([128, ST], f32, tag="outt")
                        nc.vector.scalar_tensor_tensor(
                            tmp[:], eop[:], b2_sb[:, e : e + 1], gs[:],
                            ALU.add, ALU.mult,
                        )
                        nc.vector.tensor_add(acc[:], acc[:], tmp[:])

                outt = opool.tile([128, ST], f32, tag="outt")
                nc.vector.tensor_tensor(outt[:], acc[:], rbp[:], ALU.mult)
                nc.sync.dma_start(outd[:, i * ST : (i + 1) * ST], outt[:])
                if i + 1 < nst:
                    cur_etg = next_etg
                    if n_onehot:
                        cur_masks = next_masks

    if legalize:
        _legalize_waits(nc)
    # populate .instr bytes for extended-ISA instructions (library reload for
    # dma_gather) — raw Bass skips Bacc's codegen pass; walrus errors with
    # "ISA wrong length" on empty instr otherwise
    mybir.codegen_inst_isa_subclasses(nc)
    return nc


def marshal_inputs(
    x, emb0, emb1, W1, b1, W2, b2, Wg, bg, nst=NST, ncores=NCORES, n_gather=2
):
    """Host-side: cast/reshape full inputs into per-core in_maps."""
    n_tok = ncores * nst * ST
    gathered = [t < n_gather for t in range(2)]
    tables = [emb0, emb1]

    def _wrap_idx(col):
        # dma_gather wrapped layout, tiled 8x across partitions (8 Q7 cores)
        w = (
            col[:n_tok].astype(np.int16).reshape(ncores, nst, ST // 16, 16)
            .transpose(0, 1, 3, 2)
        )
        return np.ascontiguousarray(np.tile(w, (1, 1, 8, 1)))

    def _f16_rows(col):
        return np.ascontiguousarray(
            col[:n_tok].astype(np.float16).reshape(ncores, nst, 1, ST)
        )

    shared = {}
    xh = {}
    for t in range(2):
        if gathered[t]:
            xh[f"x{t}i"] = _wrap_idx(x[:, t])
        else:
            xh[f"x{t}"] = _f16_rows(x[:, t])
    if n_gather:
        shared["embg"] = np.ascontiguousarray(
            np.stack([np.asarray(tables[t]) for t in range(2) if gathered[t]]).astype(
                BF16
            )
        )
    if n_gather < 2:
        onehot_tabs = [np.asarray(tables[t]) for t in range(2) if not gathered[t]]
        shared["embs"] = np.ascontiguousarray(
            np.stack(onehot_tabs)
            .reshape(len(onehot_tabs), VC, 128, DC, 128)
            .transpose(2, 0, 1, 3, 4)
            .astype(BF16)
        )
        shared["ivs"] = np.ascontiguousarray(
            (np.arange(VC)[None, :] * 128 + np.arange(128)[:, None]).astype(np.float32)
        )

    shared["w1s"] = np.ascontiguousarray(
        np.asarray(W1).reshape(E, KC, 128, DC, 128).transpose(0, 2, 1, 3, 4).astype(BF16)
    )
    shared["w2s"] = np.ascontiguousarray(
        np.asarray(W2).reshape(E, DC, 128, OUT).transpose(2, 0, 1, 3).astype(BF16)
    )
    shared["wgs"] = np.ascontiguousarray(
        np.asarray(Wg).reshape(KC, 128, E).transpose(1, 0, 2).astype(BF16)
    )
    shared["b1s"] = np.ascontiguousarray(
        np.asarray(b1).reshape(E, DC, 128).transpose(2, 0, 1).astype(np.float32)
    )
    shared["b2s"] = np.ascontiguousarray(np.asarray(b2).T.astype(np.float32))
    shared["bgs"] = np.ascontiguousarray(np.asarray(bg).reshape(E, 1).astype(np.float32))
    shared["sels"] = np.ascontiguousarray(
        np.broadcast_to(np.eye(E, dtype=np.float32)[:, :, None], (E, E, 128)).astype(
            BF16
        )
    )
    return [{**{k: v[c] for k, v in xh.items()}, **shared} for c in range(ncores)]


def kernel(x, emb0, emb1, W1, b1, W2, b2, Wg, bg):
    global LAST_EXEC_NS
    nc = build_program()
    in_maps = marshal_inputs(x, emb0, emb1, W1, b1, W2, b2, Wg, bg)
    trace = os.environ.get("BASSMOE_TRACE", "0") == "1"
    res = run_bass_kernel_spmd(nc, in_maps, list(range(NCORES)), trace=trace)
    LAST_EXEC_NS = res.exec_time_ns
    out = np.empty((B, OUT), dtype=np.float32)
    for c in range(NCORES):
        out[c * BL : (c + 1) * BL, :] = res.results[c]["out"].T
    return out



# revision 14
# speedup vs baseline: 2.0141x; 2.0141x over previous
"""MoE model (embed -> gate -> 4 dense experts -> softmax combine) on 8 TRN2 cores.

Key algebraic restructuring vs the naive dense pipeline: the tokens only index
V=512 distinct embedding rows per table, so the entire first-layer expert
matmul (e @ W1, 8.4 MMAC/token — 95% of the model's FLOPs) is precomputed
per *vocab entry* instead of per token:

  A0[v] = concat_e(emb0[v] @ W1[e,:1024]) (+b1)   -> [V, E*D (+gating col)]
  A1[v] = concat_e(emb1[v] @ W1[e,1024:])         -> [V, E*D (+gating col)]
  z[t]  = A0[x0[t]] + A1[x1[t]]                   (gather + add)
  out[t]= sum_e softmax_e(z_gate) * (W2[e] @ silu(z[t,e,:]) + b2)

The gating logits (e @ Wg + bg) are folded into the same tables as a 33rd
128-wide column chunk, so one gather feeds both the experts and the gate.

Per core (8192 tokens): the A tables (4.2 MB each, bf16) are built on the PE
at kernel start (fm matmul + PE transpose) and written to DRAM scratch; the
main loop gathers token rows with *non-transposing* gpsimd gather DMAs
(token-major, 1 descriptor/row — descriptor-gen stays off the critical path),
adds them on the DVE, transposes z back to feature-major on the PE (128x128
identity matmuls into PSUM), applies Silu on the scalar engine straight out
of PSUM, and runs the small W2 matmul + softmax-weighted combine as before.

bf16 tables with fp32 PSUM accumulation: rel err vs fp32 reference ~0.5%.
"""

import os
import numpy as np
import ml_dtypes

import concourse.bass as bass
import concourse.mybir as mybir
import concourse.tile as tile
from concourse.bass_utils import run_bass_kernel_spmd

BF16 = ml_dtypes.bfloat16

B = 65536
V = 512
D = 1024
IN = 2048
E = 4
OUT = 128
NCORES = 8
BL = B // NCORES          # tokens per core
ST = 256                  # tokens per supertile
NST = BL // ST            # supertiles per core
KC8 = 8                   # 128-chunks of one table-half's input dim (1024)
JE = 32                   # expert-feature chunks per table row (E*D/128)
JW = JE + 1               # + 1 gating chunk
ROWE = JW * 128           # table row length in elements (4224)
DC = D // 128

LAST_EXEC_NS = None       # set when BASSMOE_TRACE=1
LAST_RES = None


def _legalize_waits(nc, max_waits=1):
    """This walrus build rejects instructions carrying more than ~1 sync-wait
    command ("Too many sync wait commands", CoreV2/V3GenImpl setupSyncWait).
    Hoist all but the last wait of every instruction onto single-wait NoOps
    placed immediately before it in the same engine's stream."""
    for f in nc.m.functions:
        for bb in f.blocks:
            insts = bb.instructions
            if not any(
                inst.sync_info is not None and len(inst.sync_info.on_wait) > max_waits
                for inst in insts
            ):
                continue
            new = []
            for inst in insts:
                si = inst.sync_info
                waits = list(si.on_wait) if si is not None else []
                if len(waits) > max_waits:
                    for w in waits[:-max_waits]:
                        nop = mybir.InstNoOp(
                            name=f"legw-{nc.next_id()}", ins=[], outs=[]
                        )
                        nop.engine = inst.engine
                        nop.sync_info = mybir.SyncInfo(on_wait=[w], on_update=[])
                        new.append(nop)
                    inst.sync_info = mybir.SyncInfo(
                        on_wait=waits[-max_waits:], on_update=list(si.on_update)
                    )
                new.append(inst)
            bb.instructions = new


def _build_tables(nc, tc, ALU, f32, bf16, embtd, bias0d, w1d, atd, ident_sb):
    with (
        tc.tile_pool(name="procst", bufs=1) as procst,
        tc.tile_pool(name="w1st", bufs=4) as w1pool,
        tc.tile_pool(name="stg", bufs=3) as stpool,
        tc.tile_pool(name="rowt", bufs=1) as rowpool,
        tc.tile_pool(name="ppa", bufs=1, space="PSUM") as ppa,
        tc.tile_pool(name="ppt", bufs=1, space="PSUM") as ppt,
    ):
        embt_sb = procst.tile([128, 2, KC8, V], bf16)
        nc.sync.dma_start(embt_sb[:], embtd[:])
        bias0_sb = procst.tile([128, JW], f32)
        nc.sync.dma_start(bias0_sb[:], bias0d[:])

        for t in range(2):
            rows = []
            for vc in range(V // 128):
                rowt = rowpool.tile([128, ROWE], bf16, tag=f"row{vc}")
                rows.append(rowt)
            for c in range(JW):
                w1c = w1pool.tile([128, KC8, 128], bf16, tag="w1c")
                nc.sync.dma_start(w1c[:], w1d[t, c])
                psA = ppa.tile([128, V], f32, tag="pa")
                for kc in range(KC8):
                    nc.tensor.matmul(
                        psA[:],
                        w1c[:, kc, :],
                        embt_sb[:, t, kc, :],
                        start=(kc == 0),
                        stop=(kc == KC8 - 1),
                    )
                stage = stpool.tile([128, V], bf16, tag="stg")
                if t == 0:
                    nc.vector.tensor_scalar(
                        stage[:], psA[:], bias0_sb[:, c : c + 1], None, ALU.add
                    )
                else:
                    nc.vector.tensor_copy(stage[:], psA[:])
                psT = ppt.tile([128, V // 128, 128], bf16, tag="pt")
                for vc in range(V // 128):
                    nc.tensor.transpose(
                        psT[:, vc, :],
                        stage[:, vc * 128 : (vc + 1) * 128],
                        ident_sb[:],
                    )
                for vc in range(V // 128):
                    nc.scalar.copy(
                        rows[vc][:, c * 128 : (c + 1) * 128], psT[:, vc, :]
                    )
            for vc in range(V // 128):
                nc.sync.dma_start(atd[t][vc * 128 : (vc + 1) * 128, :], rows[vc][:])


def build_program(nst=NST, legalize=True):
    dt = mybir.dt
    f32, bf16 = dt.float32, dt.bfloat16
    AF = mybir.ActivationFunctionType
    ALU = mybir.AluOpType

    nc = bass.Bass()

    xd = [
        nc.dram_tensor(f"x{t}i", [nst, 128, ST // 16], dt.int16, kind="ExternalInput")
        for t in range(2)
    ]
    embtd = nc.dram_tensor("embt", [128, 2, KC8, V], bf16, kind="ExternalInput")
    w1d = nc.dram_tensor("w1s", [2, JW, 128, KC8, 128], bf16, kind="ExternalInput")
    w2d = nc.dram_tensor("w2s", [128, E, DC, OUT], bf16, kind="ExternalInput")
    b2d = nc.dram_tensor("b2s", [128, E], f32, kind="ExternalInput")
    bias0d = nc.dram_tensor("bias0", [128, JW], f32, kind="ExternalInput")
    identd = nc.dram_tensor("ident", [128, 128], bf16, kind="ExternalInput")
    seld = nc.dram_tensor("sels", [E, E, 128], bf16, kind="ExternalInput")
    outd = nc.dram_tensor("out", [128, nst * ST], f32, kind="ExternalOutput")

    # A tables, DRAM scratch (device-built, then gathered from)
    atd = [
        nc.dram_tensor(f"at{t}", [V, ROWE], bf16, kind="Internal") for t in range(2)
    ]

    with tile.TileContext(nc) as tc:
        with (
            tc.tile_pool(name="const", bufs=1) as cpool,
            tc.tile_pool(name="xt", bufs=2) as xpool,
            tc.tile_pool(name="ga", bufs=2) as apool,
            tc.tile_pool(name="zt", bufs=2) as zpool,
            tc.tile_pool(name="ht", bufs=1) as hpool,
            tc.tile_pool(name="sm", bufs=2) as smpool,
            tc.tile_pool(name="gsb", bufs=2) as gspool,
            tc.tile_pool(name="outp", bufs=2) as opool,
            tc.tile_pool(name="pzs", bufs=2, space="PSUM") as pzs,
            tc.tile_pool(name="peo", bufs=1, space="PSUM") as peo,
            tc.tile_pool(name="pgp", bufs=1, space="PSUM") as pgp,
            tc.tile_pool(name="pms", bufs=1, space="PSUM") as pms,
        ):
            from concourse import library_config

            nc.gpsimd.load_library(library_config.mlp)
            streg = nc.gpsimd.to_reg(ST)

            # --- resident constants ---
            ident_sb = cpool.tile([128, 128], bf16)
            nc.sync.dma_start(ident_sb[:], identd[:])
            w2_sb = cpool.tile([128, E, DC, OUT], bf16)
            nc.sync.dma_start(w2_sb[:], w2d[:])
            b2_sb = cpool.tile([128, E], f32)
            nc.sync.dma_start(b2_sb[:], b2d[:])
            sel_sb = cpool.tile([E, E, 128], bf16)
            nc.sync.dma_start(sel_sb[:], seld[:])
            ones4 = cpool.tile([E, 1], bf16)
            nc.vector.memset(ones4[:], 1.0)
            ones14 = cpool.tile([1, E], bf16)
            nc.vector.memset(ones14[:], 1.0)

            # --- prologue: build A tables on the PE, write to DRAM scratch ---
            _build_tables(
                nc, tc, ALU, f32, bf16, embtd, bias0d, w1d, atd, ident_sb
            )

            # --- main loop: gather -> add -> transpose -> silu -> W2 -> mix ---
            def issue_gather(i, t):
                xi = xpool.tile([128, ST // 16], dt.int16, tag=f"xi{t}")
                nc.sync.dma_start(xi[:], xd[t][i])
                at_ = apool.tile([128, ST // 128, ROWE], bf16, tag=f"a{t}")
                nc.gpsimd.dma_gather(
                    out_ap=at_[:],
                    in_ap=atd[t][:],
                    idxs_ap=xi[:],
                    num_idxs=ST,
                    num_idxs_reg=streg,
                    elem_size=ROWE,
                    transpose=False,
                )
                return at_

            cur = [issue_gather(0, t) for t in range(2)]

            for i in range(nst):
                a0, a1 = cur
                z = zpool.tile([128, ST // 128, ROWE], bf16, tag="z")
                nc.vector.tensor_tensor(z[:], a0[:], a1[:], ALU.add)

                if i + 1 < nst:
                    cur = [issue_gather(i + 1, t) for t in range(2)]

                h = hpool.tile([128, JE, ST], bf16, tag="h")
                for k in range(JE // 4):
                    zp = pzs.tile([128, 4, ST], bf16, tag="zp")
                    for j in range(4):
                        fc = 4 * k + j
                        for g in range(ST // 128):
                            nc.tensor.transpose(
                                zp[:, j, g * 128 : (g + 1) * 128],
                                z[:, g, fc * 128 : (fc + 1) * 128],
                                ident_sb[:],
                            )
                    nc.scalar.activation(h[:, 4 * k : 4 * k + 4, :], zp[:], AF.Silu)

                # gating chunk (feature-major logits on partitions 0..3)
                pg = pzs.tile([128, 4, ST], bf16, tag="zp")
                for g in range(ST // 128):
                    nc.tensor.transpose(
                        pg[:, 0, g * 128 : (g + 1) * 128],
                        z[:, g, JE * 128 : JE * 128 + 128],
                        ident_sb[:],
                    )
                expt = smpool.tile([E, ST], bf16, tag="expt")
                nc.scalar.activation(expt[:], pg[0:E, 0, :], AF.Exp)
                gsum = pms.tile([128, ST], f32, tag="gsum")
                sp = gsum[0:1, :]
                nc.tensor.matmul(sp, ones4[:], expt[:], start=True, stop=True)
                rec = smpool.tile([1, ST], f32, tag="rec")
                nc.vector.reciprocal_approx_fast(rec[:], sp)
                recb = smpool.tile([1, ST], bf16, tag="recb")
                nc.vector.tensor_copy(recb[:], rec[:])
                rb4 = gsum[32:36, :]
                nc.tensor.matmul(rb4, ones14[:], recb[:], start=True, stop=True)
                gates = smpool.tile([E, ST], bf16, tag="gates")
                nc.vector.tensor_tensor(gates[:], expt[:], rb4, ALU.mult)

                acc = opool.tile([128, ST], f32, tag="acc")
                for e in range(E):
                    eop = peo.tile([128, ST], f32, tag="eo")
                    for dc in range(DC):
                        nc.tensor.matmul(
                            eop[:],
                            w2_sb[:, e, dc, :],
                            h[:, e * DC + dc, :],
                            start=(dc == 0),
                            stop=(dc == DC - 1),
                        )
                    gp = pgp.tile([128, ST], f32, tag="gp")
                    nc.tensor.matmul(
                        gp[:], sel_sb[:, e, :], gates[:], start=True, stop=True
                    )
                    gpsb = gspool.tile([128, ST], bf16, tag="gpsb")
                    nc.vector.tensor_copy(gpsb[:], gp[:])
                    if e == 0:
                        nc.vector.scalar_tensor_tensor(
                            acc[:], eop[:], b2_sb[:, e : e + 1], gpsb[:],
                            ALU.add, ALU.mult,
                        )
                    else:
                        tmp = opool.tile([128, ST], f32, tag="tmp")
                        nc.vector.scalar_tensor_tensor(
                            tmp[:], eop[:], b2_sb[:, e : e + 1], gpsb[:],
                            ALU.add, ALU.mult,
                        )
                        nc.vector.tensor_add(acc[:], acc[:], tmp[:])
                nc.sync.dma_start(outd[:, i * ST : (i + 1) * ST], acc[:])

    if legalize:
        _legalize_waits(nc)
    mybir.codegen_inst_isa_subclasses(nc)
    return nc


def marshal_inputs(x, emb0, emb1, W1, b1, W2, b2, Wg, bg, nst=NST, ncores=NCORES):
    """Host-side: cast/reshape full inputs into per-core in_maps."""
    n_tok = ncores * nst * ST

    def _wrap_idx(col):
        w = (
            col[:n_tok].astype(np.int16).reshape(ncores, nst, ST // 16, 16)
            .transpose(0, 1, 3, 2)
        )
        return np.ascontiguousarray(np.tile(w, (1, 1, 8, 1)))

    x = np.asarray(x)
    xh = {f"x{t}i": _wrap_idx(x[:, t]) for t in range(2)}

    shared = {}
    # embt[p, t, kc, v] = emb_t[v, kc*128+p]  (partition-major, matches tile)
    embt = np.stack(
        [
            np.asarray(e).T.reshape(KC8, 128, V).transpose(1, 0, 2)
            for e in (emb0, emb1)
        ]
    ).transpose(1, 0, 2, 3)
    shared["embt"] = np.ascontiguousarray(embt.astype(BF16))

    # w1s[t, c, p, kc, j]: c<32 -> W1[e=c//8][t*1024+kc*128+p, (c%8)*128+j]
    #                      c=32 -> Wg[t*1024+kc*128+p, j] for j<4 else 0
    W1r = np.asarray(W1).reshape(E, 2, KC8, 128, DC, 128)
    w1e = np.transpose(W1r, (1, 0, 4, 3, 2, 5)).reshape(2, JE, 128, KC8, 128)
    wgt = np.zeros((2, 1, 128, KC8, 128), dtype=np.float32)
    Wgr = np.asarray(Wg).reshape(2, KC8, 128, E)
    wgt[:, 0, :, :, :E] = Wgr.transpose(0, 2, 1, 3)
    shared["w1s"] = np.ascontiguousarray(
        np.concatenate([w1e, wgt], axis=1).astype(BF16)
    )

    shared["w2s"] = np.ascontiguousarray(
        np.asarray(W2).reshape(E, DC, 128, OUT).transpose(2, 0, 1, 3).astype(BF16)
    )
    shared["b2s"] = np.ascontiguousarray(np.asarray(b2).T.astype(np.float32))

    bias0 = np.zeros((128, JW), dtype=np.float32)
    bias0[:, :JE] = np.asarray(b1).reshape(E, DC, 128).transpose(2, 0, 1).reshape(128, JE)
    bias0[:E, JE] = np.asarray(bg)
    shared["bias0"] = np.ascontiguousarray(bias0)

    shared["ident"] = np.ascontiguousarray(np.eye(128, dtype=np.float32).astype(BF16))
    shared["sels"] = np.ascontiguousarray(
        np.broadcast_to(np.eye(E, dtype=np.float32)[:, :, None], (E, E, 128)).astype(
            BF16
        )
    )
    return [{**{k: v[c] for k, v in xh.items()}, **shared} for c in range(ncores)]


def kernel(x, emb0, emb1, W1, b1, W2, b2, Wg, bg):
    global LAST_EXEC_NS, LAST_RES
    nc = build_program()
    in_maps = marshal_inputs(x, emb0, emb1, W1, b1, W2, b2, Wg, bg)
    trace = os.environ.get("BASSMOE_TRACE", "0") == "1"
    res = run_bass_kernel_spmd(nc, in_maps, list(range(NCORES)), trace=trace)
    LAST_EXEC_NS = res.exec_time_ns
    LAST_RES = res
    out = np.empty((B, OUT), dtype=np.float32)
    for c in range(NCORES):
        out[c * BL : (c + 1) * BL, :] = res.results[c]["out"].T
    return out


# revision 17
# speedup vs baseline: 2.2337x; 1.1090x over previous
"""MoE model (embed -> gate -> 4 dense experts -> softmax combine) on 8 TRN2 cores.

Key algebraic restructuring vs the naive dense pipeline: the tokens only index
V=512 distinct embedding rows per table, so the entire first-layer expert
matmul (e @ W1, 8.4 MMAC/token — 95% of the model's FLOPs) is precomputed
per *vocab entry* instead of per token:

  A0[v] = concat_e(emb0[v] @ W1[e,:1024]) (+b1)   -> [V, E*D (+gating col)]
  A1[v] = concat_e(emb1[v] @ W1[e,1024:])         -> [V, E*D (+gating col)]
  z[t]  = A0[x0[t]] + A1[x1[t]]                   (gather + add)
  out[t]= sum_e softmax_e(z_gate) * (W2[e] @ silu(z[t,e,:]) + b2)

The gating logits (e @ Wg + bg) are folded into the same tables as a 33rd
128-wide column chunk, so one gather feeds both the experts and the gate.

Per core (8192 tokens): the A tables (4.2 MB each, bf16) are built on the PE
at kernel start (fm matmul + PE transpose) and written to DRAM scratch; the
main loop gathers token rows with *non-transposing* gpsimd gather DMAs
(token-major, 1 descriptor/row — descriptor-gen stays off the critical path),
adds them on the DVE, transposes z back to feature-major on the PE (128x128
identity matmuls into PSUM), applies Silu on the scalar engine straight out
of PSUM, and runs the small W2 matmul + softmax-weighted combine as before.

bf16 tables with fp32 PSUM accumulation: rel err vs fp32 reference ~0.5%.
"""

import os
import numpy as np
import ml_dtypes

import concourse.bass as bass
import concourse.mybir as mybir
import concourse.tile as tile
from concourse.bass_utils import run_bass_kernel_spmd

BF16 = ml_dtypes.bfloat16

B = 65536
V = 512
D = 1024
IN = 2048
E = 4
OUT = 128
NCORES = 8
BL = B // NCORES          # tokens per core
ST = 256                  # tokens per supertile
NST = BL // ST            # supertiles per core
KC8 = 8                   # 128-chunks of one table-half's input dim (1024)
JE = 32                   # expert-feature chunks per table row (E*D/128)
JW = JE + 1               # + 1 gating chunk
ROWE = JW * 128           # table row length in elements (4224)
DC = D // 128

LAST_EXEC_NS = None       # set when BASSMOE_TRACE=1
LAST_RES = None


def _legalize_waits(nc, max_waits=1):
    """This walrus build rejects instructions carrying more than ~1 sync-wait
    command ("Too many sync wait commands", CoreV2/V3GenImpl setupSyncWait).
    Hoist all but the last wait of every instruction onto single-wait NoOps
    placed immediately before it in the same engine's stream."""
    for f in nc.m.functions:
        for bb in f.blocks:
            insts = bb.instructions
            if not any(
                inst.sync_info is not None and len(inst.sync_info.on_wait) > max_waits
                for inst in insts
            ):
                continue
            new = []
            for inst in insts:
                si = inst.sync_info
                waits = list(si.on_wait) if si is not None else []
                if len(waits) > max_waits:
                    for w in waits[:-max_waits]:
                        nop = mybir.InstNoOp(
                            name=f"legw-{nc.next_id()}", ins=[], outs=[]
                        )
                        nop.engine = inst.engine
                        nop.sync_info = mybir.SyncInfo(on_wait=[w], on_update=[])
                        new.append(nop)
                    inst.sync_info = mybir.SyncInfo(
                        on_wait=waits[-max_waits:], on_update=list(si.on_update)
                    )
                new.append(inst)
            bb.instructions = new


def _build_tables(nc, tc, ALU, f32, bf16, embtd, bias0d, w1d, atd, ident_sb):
    with (
        tc.tile_pool(name="procst", bufs=1) as procst,
        tc.tile_pool(name="w1st", bufs=4) as w1pool,
        tc.tile_pool(name="stg", bufs=3) as stpool,
        tc.tile_pool(name="rowt", bufs=1) as rowpool,
        tc.tile_pool(name="ppa", bufs=1, space="PSUM") as ppa,
        tc.tile_pool(name="ppt", bufs=1, space="PSUM") as ppt,
    ):
        embt_sb = procst.tile([128, 2, KC8, V], bf16)
        nc.sync.dma_start(embt_sb[:], embtd[:])
        bias0_sb = procst.tile([128, JW], f32)
        nc.sync.dma_start(bias0_sb[:], bias0d[:])

        for t in range(2):
            rows_all = rowpool.tile([128, V // 128, ROWE], bf16, tag="rows")
            for c in range(JW):
                w1c = w1pool.tile([128, KC8, 128], bf16, tag="w1c")
                nc.sync.dma_start(w1c[:], w1d[t, c])
                psA = ppa.tile([128, V], f32, tag="pa")
                for kc in range(KC8):
                    nc.tensor.matmul(
                        psA[:],
                        w1c[:, kc, :],
                        embt_sb[:, t, kc, :],
                        start=(kc == 0),
                        stop=(kc == KC8 - 1),
                    )
                stage = stpool.tile([128, V], bf16, tag="stg")
                if t == 0:
                    nc.vector.tensor_scalar(
                        stage[:], psA[:], bias0_sb[:, c : c + 1], None, ALU.add
                    )
                else:
                    nc.vector.tensor_copy(stage[:], psA[:])
                psT = ppt.tile([128, V // 128, 128], bf16, tag="pt")
                for vc in range(V // 128):
                    nc.tensor.transpose(
                        psT[:, vc, :],
                        stage[:, vc * 128 : (vc + 1) * 128],
                        ident_sb[:],
                    )
                nc.scalar.copy(
                    rows_all[:, :, c * 128 : (c + 1) * 128], psT[:]
                )
            for vc in range(V // 128):
                nc.sync.dma_start(
                    atd[t][vc * 128 : (vc + 1) * 128, :], rows_all[:, vc, :]
                )


def build_program(nst=NST, legalize=True):
    dt = mybir.dt
    f32, bf16 = dt.float32, dt.bfloat16
    AF = mybir.ActivationFunctionType
    ALU = mybir.AluOpType

    nc = bass.Bass()

    xd = [
        nc.dram_tensor(f"x{t}i", [128, nst, ST // 16], dt.int16, kind="ExternalInput")
        for t in range(2)
    ]
    embtd = nc.dram_tensor("embt", [128, 2, KC8, V], bf16, kind="ExternalInput")
    w1d = nc.dram_tensor("w1s", [2, JW, 128, KC8, 128], bf16, kind="ExternalInput")
    w2d = nc.dram_tensor("w2s", [128, E, DC, OUT], bf16, kind="ExternalInput")
    b2d = nc.dram_tensor("b2s", [128, E], f32, kind="ExternalInput")
    bias0d = nc.dram_tensor("bias0", [128, JW], f32, kind="ExternalInput")
    identd = nc.dram_tensor("ident", [128, 128], bf16, kind="ExternalInput")
    seld = nc.dram_tensor("sels", [E, E, 128], bf16, kind="ExternalInput")
    outd = nc.dram_tensor("out", [128, nst * ST], f32, kind="ExternalOutput")

    # A tables, DRAM scratch (device-built, then gathered from)
    atd = [
        nc.dram_tensor(f"at{t}", [V, ROWE], bf16, kind="Internal") for t in range(2)
    ]

    with tile.TileContext(nc) as tc:
        with (
            tc.tile_pool(name="const", bufs=1) as cpool,
            tc.tile_pool(name="xt", bufs=2) as xpool,
            tc.tile_pool(name="ga", bufs=2) as apool,
            tc.tile_pool(name="zt", bufs=2) as zpool,
            tc.tile_pool(name="ht", bufs=1) as hpool,
            tc.tile_pool(name="sm", bufs=2) as smpool,
            tc.tile_pool(name="gsb", bufs=2) as gspool,
            tc.tile_pool(name="outp", bufs=2) as opool,
            tc.tile_pool(name="pzs", bufs=2, space="PSUM") as pzs,
            tc.tile_pool(name="peo", bufs=2, space="PSUM") as peo,
            tc.tile_pool(name="pgp", bufs=1, space="PSUM") as pgp,
            tc.tile_pool(name="pms", bufs=1, space="PSUM") as pms,
        ):
            from concourse import library_config

            nc.gpsimd.load_library(library_config.mlp)
            streg = nc.gpsimd.to_reg(ST)

            # --- resident constants ---
            ident_sb = cpool.tile([128, 128], bf16)
            nc.sync.dma_start(ident_sb[:], identd[:])
            w2_sb = cpool.tile([128, E, DC, OUT], bf16)
            nc.sync.dma_start(w2_sb[:], w2d[:])
            b2_sb = cpool.tile([128, E], f32)
            nc.sync.dma_start(b2_sb[:], b2d[:])
            sel_sb = cpool.tile([E, E, 128], bf16)
            nc.sync.dma_start(sel_sb[:], seld[:])
            ones4 = cpool.tile([E, 1], bf16)
            nc.vector.memset(ones4[:], 1.0)
            ones14 = cpool.tile([1, E], bf16)
            nc.vector.memset(ones14[:], 1.0)

            # --- prologue: build A tables on the PE, write to DRAM scratch ---
            _build_tables(
                nc, tc, ALU, f32, bf16, embtd, bias0d, w1d, atd, ident_sb
            )

            # --- all gather indices resident up front (one DMA per table) ---
            xi_all = []
            for t in range(2):
                xia = cpool.tile([128, nst, ST // 16], dt.int16, tag=f"xia{t}")
                nc.sync.dma_start(xia[:], xd[t][:])
                xi_all.append(xia)

            # --- main loop: gather -> add -> transpose -> silu -> W2 -> mix ---
            def issue_gather(i, t):
                at_ = apool.tile([128, ST // 128, ROWE], bf16, tag=f"a{t}")
                nc.gpsimd.dma_gather(
                    out_ap=at_[:],
                    in_ap=atd[t][:],
                    idxs_ap=xi_all[t][:, i, :],
                    num_idxs=ST,
                    num_idxs_reg=streg,
                    elem_size=ROWE,
                    transpose=False,
                )
                return at_

            pend = [[issue_gather(0, t) for t in range(2)]]
            if nst > 1:
                pend.append([issue_gather(1, t) for t in range(2)])

            for i in range(nst):
                a0, a1 = pend.pop(0)
                z = zpool.tile([128, ST // 128, ROWE], bf16, tag="z")
                nc.vector.tensor_tensor(z[:], a0[:], a1[:], ALU.add)

                if i + 2 < nst:
                    pend.append([issue_gather(i + 2, t) for t in range(2)])

                h = hpool.tile([128, JE, ST], bf16, tag="h")
                for k in range(JE // 4):
                    zp = pzs.tile([128, 4, ST], bf16, tag="zp")
                    for j in range(4):
                        fc = 4 * k + j
                        for g in range(ST // 128):
                            nc.tensor.transpose(
                                zp[:, j, g * 128 : (g + 1) * 128],
                                z[:, g, fc * 128 : (fc + 1) * 128],
                                ident_sb[:],
                            )
                    nc.scalar.activation(h[:, 4 * k : 4 * k + 4, :], zp[:], AF.Silu)

                # gating chunk (feature-major logits on partitions 0..3)
                pg = pzs.tile([128, 4, ST], bf16, tag="zp")
                for g in range(ST // 128):
                    nc.tensor.transpose(
                        pg[:, 0, g * 128 : (g + 1) * 128],
                        z[:, g, JE * 128 : JE * 128 + 128],
                        ident_sb[:],
                    )
                expt = smpool.tile([E, ST], bf16, tag="expt")
                nc.scalar.activation(expt[:], pg[0:E, 0, :], AF.Exp)
                gsum = pms.tile([128, ST], f32, tag="gsum")
                sp = gsum[0:1, :]
                nc.tensor.matmul(sp, ones4[:], expt[:], start=True, stop=True)
                rec = smpool.tile([1, ST], f32, tag="rec")
                nc.vector.reciprocal_approx_fast(rec[:], sp)
                recb = smpool.tile([1, ST], bf16, tag="recb")
                nc.vector.tensor_copy(recb[:], rec[:])
                rb4 = gsum[32:36, :]
                nc.tensor.matmul(rb4, ones14[:], recb[:], start=True, stop=True)
                gates = smpool.tile([E, ST], bf16, tag="gates")
                nc.vector.tensor_tensor(gates[:], expt[:], rb4, ALU.mult)

                acc = opool.tile([128, ST], f32, tag="acc")
                for e in range(E):
                    eop = peo.tile([128, ST], f32, tag="eo")
                    for dc in range(DC):
                        nc.tensor.matmul(
                            eop[:],
                            w2_sb[:, e, dc, :],
                            h[:, e * DC + dc, :],
                            start=(dc == 0),
                            stop=(dc == DC - 1),
                        )
                    gp = pgp.tile([128, ST], f32, tag="gp")
                    nc.tensor.matmul(
                        gp[:], sel_sb[:, e, :], gates[:], start=True, stop=True
                    )
                    gpsb = gspool.tile([128, ST], bf16, tag="gpsb")
                    nc.vector.tensor_copy(gpsb[:], gp[:])
                    if e == 0:
                        nc.vector.scalar_tensor_tensor(
                            acc[:], eop[:], b2_sb[:, e : e + 1], gpsb[:],
                            ALU.add, ALU.mult,
                        )
                    else:
                        tmp = opool.tile([128, ST], f32, tag="tmp")
                        nc.vector.scalar_tensor_tensor(
                            tmp[:], eop[:], b2_sb[:, e : e + 1], gpsb[:],
                            ALU.add, ALU.mult,
                        )
                        nc.vector.tensor_add(acc[:], acc[:], tmp[:])
                nc.sync.dma_start(outd[:, i * ST : (i + 1) * ST], acc[:])

    if legalize:
        _legalize_waits(nc)
    mybir.codegen_inst_isa_subclasses(nc)
    return nc


def marshal_inputs(x, emb0, emb1, W1, b1, W2, b2, Wg, bg, nst=NST, ncores=NCORES):
    """Host-side: cast/reshape full inputs into per-core in_maps."""
    n_tok = ncores * nst * ST

    def _wrap_idx(col):
        w = (
            col[:n_tok].astype(np.int16).reshape(ncores, nst, ST // 16, 16)
            .transpose(0, 1, 3, 2)
        )
        w = np.tile(w, (1, 1, 8, 1))          # [c, nst, 128, 16]
        return np.ascontiguousarray(w.transpose(0, 2, 1, 3))

    x = np.asarray(x)
    xh = {f"x{t}i": _wrap_idx(x[:, t]) for t in range(2)}

    shared = {}
    # embt[p, t, kc, v] = emb_t[v, kc*128+p]  (partition-major, matches tile)
    embt = np.stack(
        [
            np.asarray(e).T.reshape(KC8, 128, V).transpose(1, 0, 2)
            for e in (emb0, emb1)
        ]
    ).transpose(1, 0, 2, 3)
    shared["embt"] = np.ascontiguousarray(embt.astype(BF16))

    # w1s[t, c, p, kc, j]: c<32 -> W1[e=c//8][t*1024+kc*128+p, (c%8)*128+j]
    #                      c=32 -> Wg[t*1024+kc*128+p, j] for j<4 else 0
    W1r = np.asarray(W1).reshape(E, 2, KC8, 128, DC, 128)
    w1e = np.transpose(W1r, (1, 0, 4, 3, 2, 5)).reshape(2, JE, 128, KC8, 128)
    wgt = np.zeros((2, 1, 128, KC8, 128), dtype=np.float32)
    Wgr = np.asarray(Wg).reshape(2, KC8, 128, E)
    wgt[:, 0, :, :, :E] = Wgr.transpose(0, 2, 1, 3)
    shared["w1s"] = np.ascontiguousarray(
        np.concatenate([w1e, wgt], axis=1).astype(BF16)
    )

    shared["w2s"] = np.ascontiguousarray(
        np.asarray(W2).reshape(E, DC, 128, OUT).transpose(2, 0, 1, 3).astype(BF16)
    )
    shared["b2s"] = np.ascontiguousarray(np.asarray(b2).T.astype(np.float32))

    bias0 = np.zeros((128, JW), dtype=np.float32)
    bias0[:, :JE] = np.asarray(b1).reshape(E, DC, 128).transpose(2, 0, 1).reshape(128, JE)
    bias0[:E, JE] = np.asarray(bg)
    shared["bias0"] = np.ascontiguousarray(bias0)

    shared["ident"] = np.ascontiguousarray(np.eye(128, dtype=np.float32).astype(BF16))
    shared["sels"] = np.ascontiguousarray(
        np.broadcast_to(np.eye(E, dtype=np.float32)[:, :, None], (E, E, 128)).astype(
            BF16
        )
    )
    return [{**{k: v[c] for k, v in xh.items()}, **shared} for c in range(ncores)]


def kernel(x, emb0, emb1, W1, b1, W2, b2, Wg, bg):
    global LAST_EXEC_NS, LAST_RES
    nc = build_program()
    in_maps = marshal_inputs(x, emb0, emb1, W1, b1, W2, b2, Wg, bg)
    trace = os.environ.get("BASSMOE_TRACE", "0") == "1"
    res = run_bass_kernel_spmd(nc, in_maps, list(range(NCORES)), trace=trace)
    LAST_EXEC_NS = res.exec_time_ns
    LAST_RES = res
    out = np.empty((B, OUT), dtype=np.float32)
    for c in range(NCORES):
        out[c * BL : (c + 1) * BL, :] = res.results[c]["out"].T
    return out
